# revision 3
# baseline (speedup 1.0000x reference)
"""Atomwise MLP + segment_sum kernel for 8 TRN2 NeuronCores.

Strategy (data-parallel over atoms, per sharding hint):
 - Host: shard x over 8 cores (125k atoms each, molecules contiguous since
   idx_m is sorted), pre-transpose each shard to feature-major [128, n] so
   the device DMAs are contiguous and matmuls need no on-device transpose.
 - Device (per core): tile over atoms; mm1 = W1^T-stationary matmul
   -> PSUM [64, T]; ScalarE silu(+b1) -> SBUF; mm2 = h-chunk-stationary
   matmul with W2 moving -> y_atom columns [128, 1]; batched lower-
   triangular matmul computes the inclusive prefix sum of y_atom within
   each 128-atom chunk; DMA the per-chunk prefixes out.
 - Host: segment sums are differences of the (chunk-offset-corrected)
   prefix at host-known segment boundaries; add b2 * segment counts.

No collectives needed: cores own disjoint atom ranges; boundary molecules
are summed on host when merging per-core partials.
"""

import numpy as np

N_CORES = 8
N_ATOMS = 1_000_000
N_PER_CORE = N_ATOMS // N_CORES  # 125_000
N_IN = 128
N_HID = 64
CHUNK = 128                      # atoms per y-column / prefix chunk
BLOCK_COLS = 128                 # chunks per prefix block (block = 16384 atoms)
TILE_T = 512                     # atoms per mm1 tile (one PSUM bank at f32)
N_PAD = 131_072                  # N_PER_CORE padded to a multiple of 16384
N_BLOCKS = N_PAD // (CHUNK * BLOCK_COLS)   # 8
G_TOTAL = N_PAD // CHUNK                   # 1024 chunks per core
TILES_PER_BLOCK = (CHUNK * BLOCK_COLS) // TILE_T  # 32
COLS_PER_TILE = TILE_T // CHUNK            # 4

_cached = {}


def _build_nc(use_bf16=True):
    from concourse import bacc, bass, mybir, tile

    dt_x = mybir.dt.bfloat16 if use_bf16 else mybir.dt.float32
    f32 = mybir.dt.float32

    nc = bacc.Bacc("TRN2", target_bir_lowering=False, debug=False)

    xT = nc.declare_dram_parameter("xT", [N_IN, N_PAD], dt_x, isOutput=False)
    w1 = nc.declare_dram_parameter("w1", [N_IN, N_HID], dt_x, isOutput=False)
    b1 = nc.declare_dram_parameter("b1", [N_HID, 1], f32, isOutput=False)
    w2 = nc.declare_dram_parameter("w2", [N_HID, 1], dt_x, isOutput=False)
    tri = nc.declare_dram_parameter("tri", [CHUNK, BLOCK_COLS], f32, isOutput=False)
    out = nc.declare_dram_parameter("out", [CHUNK, G_TOTAL], f32, isOutput=True)

    silu = mybir.ActivationFunctionType.Silu

    with tile.TileContext(nc) as tc:
        with (
            tc.tile_pool(name="const", bufs=1) as cpool,
            tc.tile_pool(name="x", bufs=4) as xpool,
            tc.tile_pool(name="h", bufs=3) as hpool,
            tc.tile_pool(name="y", bufs=2) as ypool,
            tc.tile_pool(name="po", bufs=2) as ppool,
            tc.tile_pool(name="ps_h", bufs=2, space=bass.MemorySpace.PSUM) as psh,
            tc.tile_pool(name="ps_y", bufs=2, space=bass.MemorySpace.PSUM) as psy,
            tc.tile_pool(name="ps_p", bufs=2, space=bass.MemorySpace.PSUM) as psp,
        ):
            w1_t = cpool.tile([N_IN, N_HID], dt_x)
            b1_t = cpool.tile([N_HID, 1], f32)
            w2_t = cpool.tile([N_HID, 1], dt_x)
            tri_t = cpool.tile([CHUNK, BLOCK_COLS], f32)
            nc.sync.dma_start(out=w1_t[:], in_=w1[:])
            nc.sync.dma_start(out=b1_t[:], in_=b1[:])
            nc.sync.dma_start(out=w2_t[:], in_=w2[:])
            nc.sync.dma_start(out=tri_t[:], in_=tri[:])

            for blk in range(N_BLOCKS):
                y_mat = ypool.tile([CHUNK, BLOCK_COLS], f32)
                for tt in range(TILES_PER_BLOCK):
                    g0 = blk * CHUNK * BLOCK_COLS + tt * TILE_T
                    xt = xpool.tile([N_IN, TILE_T], dt_x)
                    nc.sync.dma_start(out=xt[:], in_=xT[:, g0:g0 + TILE_T])
                    hp = psh.tile([N_HID, TILE_T], f32)
                    nc.tensor.matmul(hp[:], w1_t[:], xt[:])
                    hs = hpool.tile([N_HID, TILE_T], dt_x)
                    nc.scalar.activation(hs[:], hp[:], silu, bias=b1_t[:])
                    yp = psy.tile([CHUNK, COLS_PER_TILE], f32)
                    for c in range(COLS_PER_TILE):
                        nc.tensor.matmul(
                            yp[:, c:c + 1],
                            hs[:, c * CHUNK:(c + 1) * CHUNK],
                            w2_t[:],
                        )
                    nc.vector.tensor_copy(
                        y_mat[:, tt * COLS_PER_TILE:(tt + 1) * COLS_PER_TILE],
                        yp[:],
                    )
                pp = psp.tile([CHUNK, BLOCK_COLS], f32)
                nc.tensor.matmul(pp[:], tri_t[:], y_mat[:])
                po = ppool.tile([CHUNK, BLOCK_COLS], f32)
                nc.vector.tensor_copy(po[:], pp[:])
                nc.sync.dma_start(
                    out=out[:, blk * BLOCK_COLS:(blk + 1) * BLOCK_COLS],
                    in_=po[:],
                )

    nc.compile()
    return nc


def _get_nc(use_bf16=True):
    key = ("nc", use_bf16)
    if key not in _cached:
        _cached[key] = _build_nc(use_bf16)
    return _cached[key]


def build_in_maps(x, W1, b1, W2, use_bf16=True):
    import ml_dtypes

    np_x = ml_dtypes.bfloat16 if use_bf16 else np.float32
    tri_np = np.triu(np.ones((CHUNK, BLOCK_COLS), dtype=np.float32))
    w1_np = np.ascontiguousarray(W1, dtype=np.float32).astype(np_x)
    b1_np = np.asarray(b1, dtype=np.float32).reshape(N_HID, 1)
    w2_np = np.ascontiguousarray(W2, dtype=np.float32).reshape(N_HID, 1).astype(np_x)

    in_maps = []
    for c in range(N_CORES):
        xs = x[c * N_PER_CORE:(c + 1) * N_PER_CORE]
        xt = np.zeros((N_IN, N_PAD), dtype=np_x)
        xt[:, :N_PER_CORE] = np.ascontiguousarray(xs.T).astype(np_x)
        in_maps.append({
            "xT": xt,
            "w1": w1_np,
            "b1": b1_np,
            "w2": w2_np,
            "tri": tri_np,
        })
    return in_maps


def run_device(x, W1, b1, W2, use_bf16=True, **run_kwargs):
    """Shard + run the NEFF on 8 cores; returns (per-core P arrays, results obj)."""
    from concourse.bass_utils import run_bass_kernel_spmd

    in_maps = build_in_maps(x, W1, b1, W2, use_bf16)
    nc = _get_nc(use_bf16)
    res = run_bass_kernel_spmd(nc, in_maps, core_ids=list(range(N_CORES)),
                               **run_kwargs)
    ps = [np.asarray(res.results[c]["out"], dtype=np.float32)
          for c in range(N_CORES)]
    return ps, res


def combine_host(ps, idx_m, num_segments, b2):
    """Per-core intra-chunk prefixes -> full segment sums."""
    nseg = int(num_segments)
    y = np.zeros(nseg, dtype=np.float64)
    idx_m = np.asarray(idx_m)
    for c in range(N_CORES):
        P = ps[c]  # [CHUNK, G_TOTAL]; column g = inclusive prefix of chunk g
        chunk_sums = P[CHUNK - 1, :].astype(np.float64)
        chunk_off = np.concatenate(([0.0], np.cumsum(chunk_sums)[:-1]))
        idx_c = idx_m[c * N_PER_CORE:(c + 1) * N_PER_CORE]
        mols, starts = np.unique(idx_c, return_index=True)
        ends = np.append(starts[1:], N_PER_CORE) - 1  # inclusive run ends

        def ploc(a):
            return P[a % CHUNK, a // CHUNK].astype(np.float64) + chunk_off[a // CHUNK]

        p_end = ploc(ends)
        s_safe = np.maximum(starts - 1, 0)
        p_start = np.where(starts > 0, ploc(s_safe), 0.0)
        np.add.at(y, mols, p_end - p_start)
    b2v = float(np.asarray(b2).reshape(-1)[0])
    if b2v != 0.0:
        y += np.bincount(idx_m, minlength=nseg).astype(np.float64) * b2v
    return y.astype(np.float32)


def kernel(x, W1, b1, W2, b2, idx_m, num_segments):
    x = np.asarray(x)
    ps, _ = run_device(x, W1, b1, W2, use_bf16=True)
    return combine_host(ps, idx_m, num_segments, b2)


# revision 12
# speedup vs baseline: 2487.9167x; 2487.9167x over previous
"""Atomwise MLP + segment_sum kernel for 8 TRN2 NeuronCores.

Strategy (data-parallel over atoms, per sharding hint):
 - Host: shard x over 8 cores (125k atoms each, molecules contiguous since
   idx_m is sorted), pre-transpose each shard to feature-major [128, n] so
   the device DMAs are contiguous and matmuls need no on-device transpose.
 - Device (per core): tile over atoms; mm1 = W1^T-stationary matmul
   -> PSUM [64, T]; ScalarE silu(+b1) -> SBUF; mm2 = h-chunk-stationary
   matmul with W2 moving -> y_atom columns [128, 1]; batched lower-
   triangular matmul computes the inclusive prefix sum of y_atom within
   each 128-atom chunk; DMA the per-chunk prefixes out.
 - Host: segment sums are differences of the (chunk-offset-corrected)
   prefix at host-known segment boundaries; add b2 * segment counts.

No collectives needed: cores own disjoint atom ranges; boundary molecules
are summed on host when merging per-core partials.
"""

import numpy as np

N_CORES = 8
N_ATOMS = 1_000_000
N_PER_CORE = N_ATOMS // N_CORES  # 125_000
N_IN = 128
N_HID = 64
CHUNK = 128                      # atoms per y-column / prefix chunk
BLOCK_COLS = 128                 # chunks per prefix block (block = 16384 atoms)
MM_T = 512                       # atoms per mm1 matmul (one PSUM bank at f32 out)
SUPER_T = 1024                   # atoms per supertile (two mm1s packed on partitions)
N_PAD = 131_072                  # N_PER_CORE padded to a multiple of 16384
N_BLOCKS = N_PAD // (CHUNK * BLOCK_COLS)   # 8
G_TOTAL = N_PAD // CHUNK                   # 1024 chunks per core
SUPER_PER_BLOCK = (CHUNK * BLOCK_COLS) // SUPER_T  # 8
COLS_PER_SUPER = SUPER_T // CHUNK          # 16

_cached = {}


def _build_nc(use_bf16=True, reps=1):
    from concourse import bacc, bass, mybir, tile

    dt_x = mybir.dt.bfloat16 if use_bf16 else mybir.dt.float32
    f32 = mybir.dt.float32

    nc = bacc.Bacc("TRN2", target_bir_lowering=False, debug=False)

    xT = nc.declare_dram_parameter("xT", [N_IN, N_PAD], dt_x, isOutput=False)
    w1 = nc.declare_dram_parameter("w1", [N_IN, N_HID], dt_x, isOutput=False)
    b1 = nc.declare_dram_parameter("b1", [CHUNK, 1], f32, isOutput=False)
    w2 = nc.declare_dram_parameter("w2", [CHUNK, 1], dt_x, isOutput=False)
    tri = nc.declare_dram_parameter("tri", [CHUNK, BLOCK_COLS], f32, isOutput=False)
    out = nc.declare_dram_parameter("out", [CHUNK, G_TOTAL], f32, isOutput=True)

    silu = mybir.ActivationFunctionType.Silu

    with tile.TileContext(nc) as tc:
        with (
            tc.tile_pool(name="const", bufs=1) as cpool,
            tc.tile_pool(name="x", bufs=4) as xpool,
            tc.tile_pool(name="h", bufs=3) as hpool,
            tc.tile_pool(name="y", bufs=2) as ypool,
            tc.tile_pool(name="po", bufs=2) as ppool,
            tc.tile_pool(name="ps_h", bufs=2, space=bass.MemorySpace.PSUM) as psh,
            tc.tile_pool(name="ps_y", bufs=2, space=bass.MemorySpace.PSUM) as psy,
            tc.tile_pool(name="ps_p", bufs=2, space=bass.MemorySpace.PSUM) as psp,
        ):
            w1_t = cpool.tile([N_IN, N_HID], dt_x)
            b1_t = cpool.tile([CHUNK, 1], f32)       # b1 duplicated on both halves
            w2_t = cpool.tile([CHUNK, 1], dt_x)      # W2 duplicated on both halves
            tri_t = cpool.tile([CHUNK, BLOCK_COLS], f32)
            nc.sync.dma_start(out=w1_t[:], in_=w1[:])
            nc.sync.dma_start(out=b1_t[:], in_=b1[:])
            nc.sync.dma_start(out=w2_t[:], in_=w2[:])
            nc.sync.dma_start(out=tri_t[:], in_=tri[:])

            def body():
                for blk in range(N_BLOCKS):
                    y_mat = ypool.tile([CHUNK, BLOCK_COLS], f32)
                    for st in range(SUPER_PER_BLOCK):
                        g0 = blk * CHUNK * BLOCK_COLS + st * SUPER_T
                        xt = xpool.tile([N_IN, SUPER_T], dt_x)
                        nc.sync.dma_start(out=xt[:], in_=xT[:, g0:g0 + SUPER_T])
                        # two mm1s pack 2*MM_T atoms onto 128 psum partitions
                        hp = psh.tile([CHUNK, MM_T], f32)
                        nc.tensor.matmul(hp[0:N_HID, :], w1_t[:], xt[:, 0:MM_T])
                        nc.tensor.matmul(hp[N_HID:CHUNK, :], w1_t[:],
                                         xt[:, MM_T:SUPER_T])
                        hs = hpool.tile([CHUNK, MM_T], dt_x)
                        nc.scalar.activation(hs[:], hp[:], silu, bias=b1_t[:])
                        yp = psy.tile([CHUNK, COLS_PER_SUPER], f32)
                        for c in range(COLS_PER_SUPER):
                            half = 0 if c < COLS_PER_SUPER // 2 else N_HID
                            cc = c % (COLS_PER_SUPER // 2)
                            nc.tensor.matmul(
                                yp[:, c:c + 1],
                                hs[half:half + N_HID,
                                   cc * CHUNK:(cc + 1) * CHUNK],
                                w2_t[half:half + N_HID],
                            )
                        nc.vector.tensor_copy(
                            y_mat[:, st * COLS_PER_SUPER:(st + 1) * COLS_PER_SUPER],
                            yp[:],
                        )
                    pp = psp.tile([CHUNK, BLOCK_COLS], f32)
                    nc.tensor.matmul(pp[:], tri_t[:], y_mat[:])
                    po = ppool.tile([CHUNK, BLOCK_COLS], f32)
                    nc.vector.tensor_copy(po[:], pp[:])
                    nc.sync.dma_start(
                        out=out[:, blk * BLOCK_COLS:(blk + 1) * BLOCK_COLS],
                        in_=po[:],
                    )

            if reps == 1:
                body()
            else:
                with tc.For_i(0, reps, 1):
                    body()

    nc.compile()
    return nc


def _get_nc(use_bf16=True, reps=1):
    key = ("nc", use_bf16, reps)
    if key not in _cached:
        _cached[key] = _build_nc(use_bf16, reps)
    return _cached[key]


def build_in_maps(x, W1, b1, W2, use_bf16=True):
    import ml_dtypes

    np_x = ml_dtypes.bfloat16 if use_bf16 else np.float32
    tri_np = np.triu(np.ones((CHUNK, BLOCK_COLS), dtype=np.float32))
    w1_np = np.ascontiguousarray(W1, dtype=np.float32).astype(np_x)
    # b1 / W2 are duplicated onto both partition halves (see _build_nc)
    b1_half = np.asarray(b1, dtype=np.float32).reshape(N_HID, 1)
    b1_np = np.concatenate([b1_half, b1_half], axis=0)
    w2_half = np.asarray(W2, dtype=np.float32).reshape(N_HID, 1)
    w2_np = np.concatenate([w2_half, w2_half], axis=0).astype(np_x)

    in_maps = []
    for c in range(N_CORES):
        xs = x[c * N_PER_CORE:(c + 1) * N_PER_CORE]
        xt = np.zeros((N_IN, N_PAD), dtype=np_x)
        xt[:, :N_PER_CORE] = np.ascontiguousarray(xs.T).astype(np_x)
        in_maps.append({
            "xT": xt,
            "w1": w1_np,
            "b1": b1_np,
            "w2": w2_np,
            "tri": tri_np,
        })
    return in_maps


def run_device(x, W1, b1, W2, use_bf16=True, **run_kwargs):
    """Shard + run the NEFF on 8 cores; returns (per-core P arrays, results obj)."""
    from concourse.bass_utils import run_bass_kernel_spmd

    in_maps = build_in_maps(x, W1, b1, W2, use_bf16)
    nc = _get_nc(use_bf16)
    res = run_bass_kernel_spmd(nc, in_maps, core_ids=list(range(N_CORES)),
                               **run_kwargs)
    ps = [np.asarray(res.results[c]["out"], dtype=np.float32)
          for c in range(N_CORES)]
    return ps, res


def combine_host(ps, idx_m, num_segments, b2):
    """Per-core intra-chunk prefixes -> full segment sums."""
    nseg = int(num_segments)
    y = np.zeros(nseg, dtype=np.float64)
    idx_m = np.asarray(idx_m)
    for c in range(N_CORES):
        P = ps[c]  # [CHUNK, G_TOTAL]; column g = inclusive prefix of chunk g
        chunk_sums = P[CHUNK - 1, :].astype(np.float64)
        chunk_off = np.concatenate(([0.0], np.cumsum(chunk_sums)[:-1]))
        idx_c = idx_m[c * N_PER_CORE:(c + 1) * N_PER_CORE]
        mols, starts = np.unique(idx_c, return_index=True)
        ends = np.append(starts[1:], N_PER_CORE) - 1  # inclusive run ends

        def ploc(a):
            return P[a % CHUNK, a // CHUNK].astype(np.float64) + chunk_off[a // CHUNK]

        p_end = ploc(ends)
        s_safe = np.maximum(starts - 1, 0)
        p_start = np.where(starts > 0, ploc(s_safe), 0.0)
        np.add.at(y, mols, p_end - p_start)
    b2v = float(np.asarray(b2).reshape(-1)[0])
    if b2v != 0.0:
        y += np.bincount(idx_m, minlength=nseg).astype(np.float64) * b2v
    return y.astype(np.float32)


def kernel(x, W1, b1, W2, b2, idx_m, num_segments):
    x = np.asarray(x)
    ps, _ = run_device(x, W1, b1, W2, use_bf16=True)
    return combine_host(ps, idx_m, num_segments, b2)
